# revision 2
# baseline (speedup 1.0000x reference)
"""Causal single-head attention (B=8, T=2048, C=1024, H=64) on 8 trn2 NeuronCores.

Strategy (data-parallel over batch, one batch element per core):
  host: feed xT = x[b].T (so C is the partition/contraction dim on chip),
        wqk = [Wq/sqrt(C) | Wk] fused projection weight, wv = Wv.
  core: for each q-block of 512 tokens
          passA: psum[0:64]  = qT block   (stationary [Wq|Wk] chunk, moving xT chunk)
                 psum[64:128]= kT block   (one fused matmul per C-chunk)
          passB: vT block = Wv^T x^T      -> PE-transpose to v[s,64], append ones col
          scores^T[s,q] = kT_chunk^T-stationary x qT-moving  (K=64)
          pT = exp(scores^T) via ACT straight out of PSUM (no max subtraction:
               |scores| < ~1, exp is safe; matches softmax exactly)
          causal mask: zero cols < d, triangular-mask the diagonal 128x128 square
          out_aug^T[65, q] += v_aug[s,65]^T-stationary x pT-moving   (ones channel
               accumulates the softmax denominator)
  host: out = (out_aug[:64] / out_aug[64]).T, stack cores.

All matmuls run in float32r (full-rate on the PE at moving-N>=256, ~1e-4 rel err).
"""

import numpy as np

import concourse.bass as bass
import concourse.mybir as mybir
import concourse.tile as tile
from concourse import bacc
from concourse.bass_utils import run_bass_kernel_spmd

B, T, C, H = 8, 2048, 1024, 64
TB = 512                 # q-block width
NBLK = T // TB           # 4 q-blocks
NC = C // 128            # 8 contraction chunks
NSC = T // 128           # 16 s-chunks
HA = H + 1               # v augmented with ones column
F32 = mybir.dt.float32
FR = mybir.dt.float32r

_compiled = {}


def build_nc():
    nc = bacc.Bacc("TRN2", target_bir_lowering=False, debug=False, num_devices=8)

    xT_d = nc.dram_tensor("xT", [C, T], FR, kind="ExternalInput").ap()
    wqk_d = nc.dram_tensor("wqk", [C, 128], FR, kind="ExternalInput").ap()
    wv_d = nc.dram_tensor("wv", [C, H], FR, kind="ExternalInput").ap()
    tri_d = nc.dram_tensor("tri", [128, 128], FR, kind="ExternalInput").ap()
    ones_d = nc.dram_tensor("ones", [128, 1], FR, kind="ExternalInput").ap()
    outT_d = nc.dram_tensor("outT", [HA, T], F32, kind="ExternalOutput").ap()

    xT_r = xT_d.rearrange("(co ci) t -> ci co t", ci=128)
    wqk_r = wqk_d.rearrange("(co ci) m -> ci co m", ci=128)
    wv_r = wv_d.rearrange("(co ci) m -> ci co m", ci=128)

    with tile.TileContext(nc) as tc:
        with (
            tc.tile_pool(name="const", bufs=1) as cpool,
            tc.tile_pool(name="persist", bufs=1) as ppool,
            tc.tile_pool(name="xin", bufs=6) as xpool,
            tc.tile_pool(name="ptile", bufs=3) as pt_pool,
            tc.tile_pool(name="vtmp", bufs=2) as vtmp_pool,
            tc.tile_pool(name="outsb", bufs=2) as out_pool,
            tc.tile_pool(name="psA", bufs=1, space="PSUM") as psA_pool,
            tc.tile_pool(name="psB", bufs=1, space="PSUM") as psB_pool,
            tc.tile_pool(name="psQK", bufs=2, space="PSUM") as psQK_pool,
            tc.tile_pool(name="psO", bufs=1, space="PSUM") as psO_pool,
            tc.tile_pool(name="psVT", bufs=1, space="PSUM") as psVT_pool,
        ):
            wqk_s = cpool.tile([128, NC, 128], FR)
            wv_s = cpool.tile([128, NC, H], FR)
            tri_s = cpool.tile([128, 128], FR)
            ones_s = cpool.tile([128, 1], FR)
            ident = cpool.tile([64, 64], F32)
            nc.sync.dma_start(wqk_s[:], wqk_r[:])
            nc.sync.dma_start(wv_s[:], wv_r[:])
            nc.sync.dma_start(tri_s[:], tri_d[:])
            nc.sync.dma_start(ones_s[:], ones_d[:])
            nc.gpsimd.memset(ident[:], 0.0)
            nc.gpsimd.affine_select(
                out=ident[:],
                in_=ident[:],
                compare_op=mybir.AluOpType.not_equal,
                fill=1.0,
                base=0,
                pattern=[[-1, 64]],
                channel_multiplier=1,
            )

            qT_s = ppool.tile([64, T], FR)
            kT_s = ppool.tile([64, T], FR)
            v_s = ppool.tile([128, NSC * HA], FR)

            for i in range(NBLK):
                q0 = i * TB
                # ---- projections for this block ----
                psA = psA_pool.tile([128, TB], F32)
                psB = psB_pool.tile([64, TB], F32)
                for c in range(NC):
                    x_c = xpool.tile([128, TB], FR)
                    nc.sync.dma_start(x_c[:], xT_r[:, c, q0 : q0 + TB])
                    nc.tensor.matmul(
                        psA[:], wqk_s[:, c, :], x_c[:],
                        start=(c == 0), stop=(c == NC - 1),
                    )
                    nc.tensor.matmul(
                        psB[:], wv_s[:, c, :], x_c[:],
                        start=(c == 0), stop=(c == NC - 1),
                    )
                nc.vector.tensor_copy(qT_s[:, q0 : q0 + TB], psA[0:64, :])
                nc.vector.tensor_copy(kT_s[:, q0 : q0 + TB], psA[64:128, :])
                vT_tmp = vtmp_pool.tile([64, TB], F32)
                nc.vector.tensor_copy(vT_tmp[:], psB[:])
                for j4 in range(TB // 128):
                    sj = (TB // 128) * i + j4
                    ps_vt = psVT_pool.tile([128, 64], F32)
                    nc.tensor.transpose(
                        ps_vt[:], vT_tmp[:, j4 * 128 : (j4 + 1) * 128], ident[:]
                    )
                    nc.vector.tensor_copy(v_s[:, sj * HA : sj * HA + H], ps_vt[:])
                    nc.vector.tensor_copy(
                        v_s[:, sj * HA + H : sj * HA + HA], ones_s[:]
                    )

                # ---- attention for this q-block ----
                nsc_i = (TB // 128) * (i + 1)  # s-chunks 0..nsc_i-1 (causal)
                psO = psO_pool.tile([HA, TB], F32)
                for g in range(nsc_i // 2):  # pairs of s-chunks share one exp
                    psQK = psQK_pool.tile([128, 1024], F32)
                    for h2 in range(2):
                        j = 2 * g + h2
                        nc.tensor.matmul(
                            psQK[:, h2 * 512 : (h2 + 1) * 512],
                            kT_s[:, j * 128 : (j + 1) * 128],
                            qT_s[:, q0 : q0 + TB],
                            start=True, stop=True,
                        )
                    pT = pt_pool.tile([128, 1024], FR)
                    nc.scalar.activation(
                        pT[:], psQK[:], mybir.ActivationFunctionType.Exp
                    )
                    for h2 in range(2):
                        j = 2 * g + h2
                        pj = pT[:, h2 * 512 : (h2 + 1) * 512]
                        d = j * 128 - q0
                        if d >= 0:  # diagonal-band chunk: apply causal mask
                            if d > 0:
                                nc.vector.memset(pj[:, 0:d].bitcast(mybir.dt.uint32), 0)
                            nc.vector.tensor_mul(
                                pj[:, d : d + 128], pj[:, d : d + 128], tri_s[:]
                            )
                        nc.tensor.matmul(
                            psO[:],
                            v_s[:, j * HA : (j + 1) * HA],
                            pj,
                            start=(j == 0), stop=(j == nsc_i - 1),
                        )
                out_sb = out_pool.tile([HA, TB], F32)
                nc.vector.tensor_copy(out_sb[:], psO[:])
                nc.sync.dma_start(outT_d[:, q0 : q0 + TB], out_sb[:])

    nc.compile()
    return nc


def _get_nc():
    if "nc" not in _compiled:
        _compiled["nc"] = build_nc()
    return _compiled["nc"]


def make_in_maps(x, Wk, Wq, Wv):
    x = np.asarray(x, dtype=np.float32)
    Wk = np.asarray(Wk, dtype=np.float32)
    Wq = np.asarray(Wq, dtype=np.float32)
    Wv = np.asarray(Wv, dtype=np.float32)
    scale = np.float32(1.0 / np.sqrt(np.float32(C)))
    wqk = np.concatenate([Wq * scale, Wk], axis=1)  # [C, 128]
    tri = np.triu(np.ones((128, 128), dtype=np.float32))
    ones = np.ones((128, 1), dtype=np.float32)
    in_maps = []
    for b in range(B):
        in_maps.append(
            {
                "xT": np.ascontiguousarray(x[b].T),
                "wqk": wqk,
                "wv": Wv,
                "tri": tri,
                "ones": ones,
            }
        )
    return in_maps


def postprocess(results):
    outs = []
    for b in range(B):
        outT = results[b]["outT"]  # [65, T]
        out = (outT[:H] / outT[H : H + 1]).T  # [T, H]
        outs.append(out)
    return np.stack(outs).astype(np.float32)


def run(x, Wk, Wq, Wv, trace=False, **kw):
    nc = _get_nc()
    in_maps = make_in_maps(x, Wk, Wq, Wv)
    res = run_bass_kernel_spmd(
        nc, in_maps, core_ids=list(range(B)), trace=trace, **kw
    )
    return postprocess(res.results), res


def kernel(x, Wk, Wq, Wv):
    out, _ = run(x, Wk, Wq, Wv, trace=False)
    return out
